# revision 19
# baseline (speedup 1.0000x reference)
"""Causal self-attention (B=2, S=2048, E=1024, H=16, D=64) on 8 TRN2 cores.

Sharding: core c = (batch b = c//4, head-group g = c%4) owns batch b and
heads 4g..4g+3 (a 256-wide slice of the QKV projections / Wo rows).
Each core computes its partial out-projection y_c = attout_c @ Wo_c; the
host sums the 4 partials per batch and adds bo (the tensor-parallel
out-proj all-reduce, done on host since cores are independent).

All device inputs/outputs are host-packed into [128, X] layouts whose
per-partition rows are contiguous in DRAM, so every DMA is 128 large
contiguous descriptors.

Device pipeline (per core), bf16 operands / fp32 PSUM accumulation:
  Q^T, K^T [256, S] via lhsT=W chunk, rhs=xT chunk; biases folded into
  the accumulation as rank-1 (bias-row x ones) matmuls so the
  PSUM->SBUF copies are pure.
  V natural [S, 4*(64+1)] with a ones column per head (softmax denom).
  scores^T [k, q] per head pair, exp on ACT (scale 1/8 folded; one
  strided call covers both heads incl. the narrowed diagonal), causal
  mask multiply on DVE (diagonal blocks only), attout^T [65, q] PV
  matmul with row 64 = denominator, normalize via
  reciprocal_approx_fast + gpsimd partition_broadcast + DVE mul.
  y = attoutT_norm.T @ Wo_c staged in SBUF as bf16, DMA'd per row-tile.

Emission is software-pipelined: the projections for q-chunk qc+1 and the
out-projection for q-chunk qc-1 are emitted as filler work between the
k-tile iterations of attention(qc), so the PE never idles waiting on the
exp/mask chain. Per-qc activations live in separate tiles to avoid
false dependencies.
"""

import numpy as np

B, S, E, H = 2, 2048, 1024, 16
D = E // H          # 64
NCORES = 8
HPC = 4             # heads per core
HD = HPC * D        # 256 cols per core
KT = E // 128       # 8 contraction tiles for projections
QC = S // 512       # 4 query chunks
NQT = S // 128      # 16 row tiles
VW = HPC * (D + 1)  # 260: V + ones column per head

_prog = None
LAST_RESULTS = None


def _build_program():
    import concourse.mybir as mybir
    import concourse.tile as tile
    from concourse import bacc

    f32 = mybir.dt.float32
    bf16 = mybir.dt.bfloat16
    fp8 = mybir.dt.float8e4
    Exp = mybir.ActivationFunctionType.Exp
    DR = mybir.MatmulPerfMode.DoubleRow
    from concourse.alu_op_type import AluOpType

    nc = bacc.Bacc(trn_type="TRN2", target_bir_lowering=False, debug=False)

    # x / Wq / Wk / Wv ship as fp8e4m3 in DoubleRow pair layout: 4 super-
    # tiles t of 256 e-rows, logical row e = 256t + 2p + s for partition p,
    # slot s. Single-pass fp8 quantization fails the error gate, so each
    # projection runs three DoubleRow passes (hi*hi + lo*hi + hi*lo, the
    # lo*lo term is negligible) that all accumulate at a common scale 8192
    # in PSUM: x1=fp8(8x), xl=fp8(64*(8x-x1)), w1=fp8(16W), w1c=fp8(1024W)
    # (exponent-shifted copy of w1), wl=fp8(64*(16W-w1)). The 1/8192 and
    # the bias fold into the PSUM->SBUF copy. Still 25% cheaper on PE than
    # bf16 at bf16-level accuracy.
    x1 = nc.dram_tensor("x1", [128, QC * 4 * 2 * 512], fp8, kind="ExternalInput").ap()
    xl = nc.dram_tensor("xl", [128, QC * 4 * 2 * 512], fp8, kind="ExternalInput").ap()
    wdr = {
        n: nc.dram_tensor(n, [128, 4 * 2 * HD], fp8, kind="ExternalInput").ap()
        for w in ("wq", "wk", "wv")
        for n in (w + "1", w + "1c", w + "l")
    }
    wo = nc.dram_tensor("wo", [128, 2 * E], bf16, kind="ExternalInput").ap()
    bqk = nc.dram_tensor("bqk", [128, 4], f32, kind="ExternalInput").ap()
    bvb = nc.dram_tensor("bvb", [128, HD], bf16, kind="ExternalInput").ap()
    mask = nc.dram_tensor("mask", [128, 4 * 2 * 512], bf16, kind="ExternalInput").ap()
    y = nc.dram_tensor("y", [128, NQT * E], bf16, kind="ExternalOutput").ap()

    with tile.TileContext(nc) as tc:
        with (
            tc.tile_pool(name="consts", bufs=1) as consts,
            tc.tile_pool(name="exps", bufs=4) as exps,
            tc.tile_pool(name="small", bufs=4) as small,
            tc.tile_pool(name="ps_mix", bufs=2, space="PSUM") as ps_mix,
            tc.tile_pool(name="ps_sc", bufs=2, space="PSUM") as ps_sc,
            tc.tile_pool(name="ps_acc", bufs=2, space="PSUM") as ps_acc,
        ):
            from concourse import library_config

            nc.gpsimd.load_library(library_config.attn)

            # ---- persistent tiles (per-qc to avoid false deps) ----
            xt_sb = [
                consts.tile([128, 4, 2, 512], fp8, name=f"xt{i}") for i in range(QC)
            ]
            xl_sb = [
                consts.tile([128, 4, 2, 512], fp8, name=f"xl{i}") for i in range(QC)
            ]
            w_sb = {
                n: consts.tile([128, 4, 2, HD], fp8, name=f"w{n}")
                for n in wdr
            }
            wo_sb = consts.tile([128, 2, E], bf16)
            mask_sb = consts.tile([128, 4, 2, 512], bf16)
            bqk_sb = consts.tile([128, 4], f32)
            bv_sb = consts.tile([128, HD], bf16)

            qt_sb = [consts.tile([128, 2, 512], bf16, name=f"qt{i}") for i in range(QC)]
            kt_sb = [consts.tile([128, 2, 512], bf16, name=f"kt{i}") for i in range(QC)]
            v_sb = [consts.tile([128, 4, VW], bf16, name=f"v{i}") for i in range(QC)]
            at_sb = [consts.tile([128, 2, 512], bf16, name=f"at{i}") for i in range(QC)]
            y_sb = [consts.tile([128, 4, E], bf16, name=f"ysb{i}") for i in range(QC)]

            # ---- DMA order tuned so qc=0 work starts ASAP ----
            def wload(n):
                nc.sync.dma_start(
                    out=w_sb[n],
                    in_=wdr[n].rearrange("p (t s c) -> p t s c", t=4, s=2),
                )

            def load_x(dst, src, qc, lo=0, hi=4):
                nc.sync.dma_start(
                    out=dst[qc][:, lo:hi],
                    in_=src[:, qc * 4096 + lo * 1024 : qc * 4096 + hi * 1024]
                    .rearrange("p (t s c) -> p t s c", t=hi - lo, s=2),
                )

            wload("wq1c")
            load_x(xt_sb, x1, 0, 0, 2)
            wload("wql")
            load_x(xt_sb, x1, 0, 2, 4)
            wload("wq1")
            load_x(xl_sb, xl, 0)
            for n in ("wk1c", "wkl", "wk1"):
                wload(n)
            nc.sync.dma_start(out=bqk_sb, in_=bqk)
            nc.sync.dma_start(out=bv_sb, in_=bvb)
            nc.sync.dma_start(
                out=mask_sb, in_=mask.rearrange("p (t j c) -> p t j c", t=4, j=2)
            )
            for n in ("wv1c", "wvl", "wv1"):
                wload(n)
            load_x(xt_sb, x1, 1)
            load_x(xl_sb, xl, 1)
            nc.sync.dma_start(out=wo_sb, in_=wo.rearrange("p (kt c) -> p kt c", kt=2))
            load_x(xt_sb, x1, 2)
            load_x(xl_sb, xl, 2)
            load_x(xt_sb, x1, 3)
            load_x(xl_sb, xl, 3)
            for qc in range(QC):
                nc.vector.memset(
                    v_sb[qc].rearrange("p rt (h c) -> p rt h c", h=HPC)[
                        :, :, :, D : D + 1
                    ],
                    1.0,
                )

            # ---- filler thunks ----
            # pass list: (x tile, w suffix); xl pass last so its DMA can
            # arrive latest. All accumulate at scale 8192 in PSUM.
            PASSES = (("1c", xt_sb), ("l", xt_sb), ("1", xl_sb))

            def qk_group(qc, wn, dst, boff, mt):
                def emit():
                    ps = ps_mix.tile([128, 512], f32, tag="mix", name=f"pqk{qc}{boff}{mt}")
                    for pi, (suf, xs) in enumerate(PASSES):
                        wt = w_sb[wn + suf]
                        for t in range(4):
                            nc.tensor.matmul(
                                ps,
                                lhsT=wt[:, t, :, mt * 128 : mt * 128 + 128],
                                rhs=xs[qc][:, t],
                                start=(pi == 0 and t == 0),
                                stop=(pi == 2 and t == 3),
                                perf_mode=DR,
                            )
                    nc.vector.tensor_scalar(
                        dst[:, mt, :],
                        ps,
                        1.0 / 8192.0,
                        bqk_sb[:, boff + mt : boff + mt + 1],
                        AluOpType.mult,
                        AluOpType.add,
                    )
                return emit

            def v_group(qc, half):
                def emit():
                    ps = ps_mix.tile([128, 512], f32, tag="mix", name=f"pv{qc}{half}")
                    for j in range(2):
                        rl = half * 2 + j
                        o = ps[:, j * 256 : j * 256 + 256]
                        for pi, (suf, xs) in enumerate(PASSES):
                            wt = w_sb["wv" + suf]
                            for t in range(4):
                                nc.tensor.matmul(
                                    o,
                                    lhsT=xs[qc][:, t, :, rl * 128 : rl * 128 + 128],
                                    rhs=wt[:, t],
                                    start=(pi == 0 and t == 0),
                                    stop=(pi == 2 and t == 3),
                                    perf_mode=DR,
                                )
                        nc.vector.scalar_tensor_tensor(
                            v_sb[qc][:, rl]
                            .rearrange("p (h c) -> p h c", h=HPC)[:, :, 0:D],
                            o.rearrange("p (h c) -> p h c", h=HPC),
                            1.0 / 8192.0,
                            bv_sb.rearrange("p (h c) -> p h c", h=HPC),
                            AluOpType.mult,
                            AluOpType.add,
                        )
                return emit

            def proj_thunks(qc):
                return [
                    qk_group(qc, "wq", qt_sb[qc], 0, 0),
                    qk_group(qc, "wk", kt_sb[qc], 2, 0),
                    v_group(qc, 0),
                    qk_group(qc, "wq", qt_sb[qc], 0, 1),
                    qk_group(qc, "wk", kt_sb[qc], 2, 1),
                    v_group(qc, 1),
                ]

            def outproj_unit(qc, qtl, nh):
                def emit():
                    ps = ps_mix.tile([128, 512], f32, tag="mix", name=f"py{qc}{qtl}{nh}")
                    for kt2 in range(2):
                        nc.tensor.matmul(
                            ps,
                            lhsT=at_sb[qc][:, kt2, qtl * 128 : qtl * 128 + 128],
                            rhs=wo_sb[:, kt2, nh * 512 : nh * 512 + 512],
                            start=(kt2 == 0),
                            stop=(kt2 == 1),
                        )
                    dst = y_sb[qc][:, qtl, nh * 512 : nh * 512 + 512]
                    qt = qc * 4 + qtl
                    if qc == 3:
                        # tail: both Act and DVE are idle; alternate so the
                        # copy chain halves, and DMA each half out as soon
                        # as it lands.
                        if nh == 0:
                            nc.scalar.copy(dst, ps)
                        else:
                            nc.vector.tensor_copy(dst, ps)
                        nc.sync.dma_start(
                            out=y[:, qt * E + nh * 512 : qt * E + nh * 512 + 512],
                            in_=dst,
                        )
                    else:
                        nc.vector.tensor_copy(dst, ps)
                        if nh == 1:
                            nc.sync.dma_start(
                                out=y[:, qt * E : (qt + 1) * E], in_=y_sb[qc][:, qtl]
                            )
                return emit

            def outproj_thunks(qc):
                return [
                    outproj_unit(qc, qtl, nh) for qtl in range(4) for nh in range(2)
                ]

            # ---- attention with interleaved fillers ----
            def attn(qc, fillers):
                nkt = 4 * (qc + 1)
                iters = 2 * nkt
                total = len(fillers)
                done = 0
                it = 0
                for mt in range(2):
                    acc = [
                        ps_acc.tile([128, 512], f32, tag="acc", name=f"acc{qc}{mt}{j}")
                        for j in range(2)
                    ]
                    for kt in range(nkt):
                        t = kt - 4 * qc
                        off = 128 * t if t > 0 else 0
                        ps = ps_sc.tile([128, 2, 512], f32, tag="sc", name=f"ps_s{kt}")
                        for j in range(2):
                            pb = j * 64
                            nc.tensor.matmul(
                                ps[:, j, off:512],
                                lhsT=kt_sb[kt // 4][
                                    pb : pb + 64, mt, (kt % 4) * 128 : (kt % 4) * 128 + 128
                                ],
                                rhs=qt_sb[qc][pb : pb + 64, mt, off:512],
                                start=True,
                                stop=True,
                            )
                        it += 1
                        while done < (total * it) // iters:
                            fillers[done]()
                            done += 1
                        ex = exps.tile([128, 2, 512], bf16, tag="ex", name=f"ex{kt}")
                        nc.scalar.activation(
                            ex[:, :, off:512], ps[:, :, off:512], Exp, scale=0.125
                        )
                        if t >= 0:
                            nc.vector.tensor_mul(
                                ex[:, :, off:512],
                                ex[:, :, off:512],
                                mask_sb[:, t, :, off:512],
                            )
                        for j in range(2):
                            h = 2 * mt + j
                            nc.tensor.matmul(
                                acc[j][0:65, off:512],
                                lhsT=v_sb[kt // 4][:, kt % 4, h * 65 : h * 65 + 65],
                                rhs=ex[:, j, off:512],
                                start=(kt == 0),
                                stop=(kt == nkt - 1),
                            )
                    # normalize this head pair
                    for j in range(2):
                        dn = small.tile([1, 512], f32, tag="dn", name=f"dn{j}")
                        # reciprocal_approx_fast misreads PSUM on HW; bounce
                        # the denominator row through SBUF first. In the
                        # tail (qc=3) Act is idle, so bounce there.
                        if qc == 3:
                            nc.scalar.copy(dn, acc[j][64:65, :])
                        else:
                            nc.vector.tensor_copy(dn, acc[j][64:65, :])
                        rc = small.tile([1, 512], f32, tag="rc", name=f"rc{j}")
                        nc.vector.reciprocal_approx_fast(out=rc, in_=dn)
                        bc = small.tile([64, 512], f32, tag="bc", name=f"bc{j}")
                        nc.gpsimd.partition_broadcast(out_ap=bc, in_ap=rc)
                        pb = j * 64
                        nc.vector.tensor_mul(
                            at_sb[qc][pb : pb + 64, mt, :], acc[j][0:64, :], bc
                        )
                while done < total:
                    fillers[done]()
                    done += 1

            # ---- main schedule ----
            for th in proj_thunks(0):
                th()
            attn(0, proj_thunks(1))
            attn(1, proj_thunks(2) + outproj_thunks(0))
            attn(2, proj_thunks(3) + outproj_thunks(1))
            attn(3, outproj_thunks(2))
            for th in outproj_thunks(3):
                th()

    nc.compile()
    return nc


def _get_program():
    global _prog
    if _prog is None:
        _prog = _build_program()
    return _prog


def _make_mask():
    import ml_dtypes

    k = np.arange(128)[:, None]
    q = np.arange(512)[None, :]
    m = np.stack([(q >= k + 128 * t) for t in range(4)])  # [4, 128, 512]
    m2 = np.repeat(m[:, None], 2, axis=1)                 # [4, 2, 128, 512]
    return np.ascontiguousarray(
        m2.transpose(2, 0, 1, 3).reshape(128, 4 * 2 * 512)
    ).astype(ml_dtypes.bfloat16)


def _pack_rows(a, ktiles):
    """[ktiles*128, C] -> [128, ktiles*C] with per-partition contiguous rows."""
    kt, c = ktiles, a.shape[1]
    return np.ascontiguousarray(
        a.reshape(kt, 128, c).transpose(1, 0, 2).reshape(128, kt * c)
    )


def _dr_layout(w):
    """[1024, C] -> [128, 4*2*C] DoubleRow pair layout, e = 256t+2p+s."""
    c = w.shape[1]
    return np.ascontiguousarray(
        w.reshape(4, 128, 2, c).transpose(1, 0, 2, 3).reshape(128, 4 * 2 * c)
    )


def _hi_lo(a):
    """fp8 residual split: a1 = fp8(a), al = fp8(64*(a - a1))."""
    import ml_dtypes

    f8 = ml_dtypes.float8_e4m3
    a1 = a.astype(f8)
    al = ((a - a1.astype(np.float32)) * 64.0).astype(f8)
    return a1, al


def _core_inputs(x, Wq, bq, Wk, bk, Wv, bv, Wo, mask, c):
    import ml_dtypes

    bf16 = ml_dtypes.bfloat16
    f8 = ml_dtypes.float8_e4m3
    b, g = divmod(c, 4)
    sl = slice(g * HD, (g + 1) * HD)
    xT = x[b].T  # [E, S]
    x_pack = np.ascontiguousarray(
        (8.0 * xT)
        .reshape(4, 128, 2, QC, 512)
        .transpose(1, 3, 0, 2, 4)
        .reshape(128, QC * 4 * 2 * 512)
    )
    x1, xlo = _hi_lo(x_pack)
    out = {"x1": x1, "xl": xlo}
    for name, W in (("wq", Wq), ("wk", Wk), ("wv", Wv)):
        w16 = _dr_layout(16.0 * W[:, sl])
        w1, wl = _hi_lo(w16)
        out[name + "1"] = w1
        out[name + "1c"] = (w1.astype(np.float32) * 64.0).astype(f8)
        out[name + "l"] = wl
    bqk = np.stack(
        [bq[sl].reshape(2, 128), bk[sl].reshape(2, 128)], axis=0
    )  # [2(qk), 2(mt), 128]
    out.update({
        "wo": _pack_rows(Wo[sl, :], 2).astype(bf16),
        "bqk": np.ascontiguousarray(bqk.reshape(4, 128).T).astype(np.float32),
        "bvb": np.ascontiguousarray(np.broadcast_to(bv[sl], (128, HD))).astype(bf16),
        "mask": mask,
    })
    return out


def _unpack_y(y_p):
    """[128, NQT*E] bf16 -> [S, E] f32"""
    return y_p.astype(np.float32).reshape(128, NQT, E).transpose(1, 0, 2).reshape(S, E)


def kernel(x, Wq, bq, Wk, bk, Wv, bv, Wo, bo, **_run_kwargs):
    from concourse.bass_utils import run_bass_kernel_spmd

    x = np.asarray(x, dtype=np.float32)
    Wq, bq = np.asarray(Wq, np.float32), np.asarray(bq, np.float32)
    Wk, bk = np.asarray(Wk, np.float32), np.asarray(bk, np.float32)
    Wv, bv = np.asarray(Wv, np.float32), np.asarray(bv, np.float32)
    Wo, bo = np.asarray(Wo, np.float32), np.asarray(bo, np.float32)

    nc = _get_program()
    mask = _make_mask()
    in_maps = [
        _core_inputs(x, Wq, bq, Wk, bk, Wv, bv, Wo, mask, c) for c in range(NCORES)
    ]
    res = run_bass_kernel_spmd(nc, in_maps, list(range(NCORES)), **_run_kwargs)
    global LAST_RESULTS
    LAST_RESULTS = res
    parts = [_unpack_y(res.results[c]["y"]) for c in range(NCORES)]
    out = np.empty((B, S, E), np.float32)
    for b in range(B):
        out[b] = parts[4 * b] + parts[4 * b + 1] + parts[4 * b + 2] + parts[4 * b + 3]
        out[b] += bo
    return out
